# revision 1
# baseline (speedup 1.0000x reference)
"""Trainium2 Bass kernel for nn_Affinity_Propagate (8-neighbor gated stencil).

Algorithm (per batch image, one image per NeuronCore):
    gate_wb[c] = shift_c(guidance[c])           (SPN shift, zero pad)
    A          = max(sum_c |gate_wb[c]|, 1e-6)
    w[c]       = gate_wb[c] / A
    gs         = sum_c w[c]
    base       = (1 - gs) * blur
    r          = blur
    repeat prop_time times:
        r = sum_c w[c] * shift_c(r) + base

Sharding: pure data parallelism, one batch image per NeuronCore (8 cores).

Layout: image rows are distributed over 120 SBUF partitions (4 rows each).
The r state is stored per-partition as 6 row slots x 642 columns (1-row/col
zero halos), so all 8 neighbor reads are plain free-dim AP offsets. Row
halos across partitions are refreshed each iteration with two small
partition-shifted SBUF->SBUF DMAs (HWDGE). Per iteration: 8 DVE
tensor-tensor products (written as float32r), 72 bank-aligned identity
matmuls (float32r rhs = 1 cyc/row) accumulating base + 8 products into
PSUM in fp32, and 4 per-row-slot ScalarE copies PSUM->SBUF that pipeline
right behind the PE; the first two products of the next iteration are
row-split and interleaved so the DVE starts the moment each copied row
lands. Measured ~300-420 us for the 24-iteration loop, absmax relative
error ~3e-4 vs the fp32 reference (float32r product rounding; full-fp32
CoreSim error is ~2e-7).
"""

import numpy as np
from contextlib import ExitStack

import concourse.bass as bass
import concourse.bacc as bacc
import concourse.tile as tile
import concourse.mybir as mybir
from concourse.bass_utils import run_bass_kernel_spmd

H, W = 480, 640
B = 8
NCORES = 8
RP = 4                  # image rows per partition
P = H // RP             # 120 partitions used
NROW = RP + 2           # row slots incl. top/bottom halo
ROWW = W + 2            # row width incl. left/right halo
FLAT = RP * W           # 2560 tight free elems per partition
OFFSETS = [(-1, -1), (-1, 0), (-1, 1), (0, -1), (0, 1), (1, -1), (1, 0), (1, 1)]
EPS = 1e-6

F32 = mybir.dt.float32
F32R = mybir.dt.float32r
ALU = mybir.AluOpType

# product emission order: dy==0 first (no row-halo dep), then dy==+1 (down
# halo, written first), then dy==-1 (up halo, written last)
ORDERED_C = [3, 4, 5, 6, 7, 0, 1, 2]


def _add_free_dim(ap, axis, step, count):
    """Insert a free dim [step, count] at the given axis of an AP (overlapping
    window reads are fine for inputs)."""
    ap = ap.unsqueeze(axis).copy()
    ap.ap[axis] = [step, count]
    return ap


def _load_shifted(nc, g_view, src2d, dy, dx):
    """DMA guidance channel (as [H, W] dram AP) shifted by (dy, dx) into the
    tight gate tile view g_view ([P, RP, W]); border elements are left
    untouched (pre-zeroed)."""
    r0 = max(0, -dy)            # dest flat row range [r0, r1)
    r1 = H - max(0, dy)
    x0 = max(0, -dx)            # dest col range [x0, x1)
    x1 = W - max(0, dx)
    p_start = (r0 + RP - 1) // RP
    p_end = r1 // RP
    # bulk partitions: all RP row slots in range
    if p_end > p_start:
        src = src2d[RP * p_start + dy:RP * p_end + dy, x0 + dx:x1 + dx]
        src = src.rearrange("(p j) w -> p j w", j=RP)
        nc.sync.dma_start(out=g_view[p_start:p_end, :, x0:x1], in_=src)
    # head partition with partial slots
    if r0 % RP != 0:
        p = p_start - 1
        j0 = r0 - RP * p
        src = src2d[RP * p + j0 + dy:RP * (p + 1) + dy, x0 + dx:x1 + dx]
        src = src.rearrange("(p j) w -> p j w", j=RP - j0)
        nc.sync.dma_start(out=g_view[p:p + 1, j0:RP, x0:x1], in_=src)
    # tail partition with partial slots
    if r1 % RP != 0:
        p = p_end
        j1 = r1 - RP * p
        src = src2d[RP * p + dy:RP * p + j1 + dy, x0 + dx:x1 + dx]
        src = src.rearrange("(p j) w -> p j w", j=j1)
        nc.sync.dma_start(out=g_view[p:p + 1, 0:j1, x0:x1], in_=src)


def _emit(ctx, tc, guid, blur, ident_d, out_d, prop_time, repeat=1,
          gp_products=0, fuse_groups=False, act_chunked=True,
          split_first=2, n_products=8, pre_repeat=1, split_last=False,
          prod_bufs=6):
    nc = tc.nc

    const_pool = ctx.enter_context(tc.tile_pool(name="const", bufs=1))
    r_pool = ctx.enter_context(tc.tile_pool(name="rbuf", bufs=1))

    ident = const_pool.tile([P, P], F32R, tag="ident", name="ident_sb")
    nc.sync.dma_start(out=ident[:], in_=ident_d)

    if fuse_groups:
        # gates packed per dy-group so each group's products are one DVE op
        grp_tiles = [
            const_pool.tile([P, 2 * FLAT], F32, tag="gB", name="gB_sb"),  # c3,c4
            const_pool.tile([P, 3 * FLAT], F32, tag="gC", name="gC_sb"),  # c5,c6,c7
            const_pool.tile([P, 3 * FLAT], F32, tag="gA", name="gA_sb"),  # c0,c1,c2
        ]
        GROUPS = [([3, 4], grp_tiles[0]), ([5, 6, 7], grp_tiles[1]),
                  ([0, 1, 2], grp_tiles[2])]
        gates = [None] * 8
        for chans, gt in GROUPS:
            for ci, c in enumerate(chans):
                gates[c] = gt[:, ci * FLAT:(ci + 1) * FLAT]
    else:
        gates = [const_pool.tile([P, FLAT], F32, tag=f"g{c}", name=f"g{c}_sb")
                 for c in range(8)]
    base = const_pool.tile([P, FLAT], F32R, tag="base", name="base_sb")

    rbufs = [r_pool.tile([P, NROW * ROWW], F32, tag=f"r{i}", name=f"r{i}_sb") for i in range(2)]
    for rb in rbufs:
        nc.gpsimd.memset(rb[:], 0.0)

    rviews = [rb[:].rearrange("p (r w) -> p r w", r=NROW) for rb in rbufs]

    # blur -> r0 center, then initial row-halo exchange
    blur_t = blur.rearrange("(p j) w -> p j w", j=RP)
    rv0 = rviews[0]
    nc.sync.dma_start(out=rv0[:, 1:1 + RP, 1:1 + W], in_=blur_t)
    nc.sync.dma_start(out=rv0[1:P, 0:1, :], in_=rv0[0:P - 1, RP:RP + 1, :])
    nc.sync.dma_start(out=rv0[0:P - 1, RP + 1:RP + 2, :], in_=rv0[1:P, 1:2, :])

    # guidance channels, SPN-shifted at load time
    gviews = []
    for c, (dy, dx) in enumerate(OFFSETS):
        gv = gates[c][:].rearrange("p (j w) -> p j w", j=RP)
        gviews.append(gv)
        # zero the border stripes the shifted load skips (full-partition APs —
        # compute ops must start at partition 0; the load overwrites the rest)
        if dy == -1:
            nc.vector.memset(gv[:, 0:1, :], 0.0)
        elif dy == 1:
            nc.vector.memset(gv[:, RP - 1:RP, :], 0.0)
        if dx == -1:
            nc.vector.memset(gv[:, :, 0:1], 0.0)
        elif dx == 1:
            nc.vector.memset(gv[:, :, W - 1:W], 0.0)
        _load_shifted(nc, gv, guid[c], dy, dx)

    # ---- gate normalization (one-time; idempotent, pre_repeat for timing) ----
    with tc.tile_pool(name="pretmp", bufs=1) as tmp_pool:
      for _pre in range(pre_repeat):
        A = tmp_pool.tile([P, FLAT], F32, tag="absum", name="absum_sb")
        S = tmp_pool.tile([P, FLAT], F32, tag="rawsum", name="rawsum_sb")
        # A = sum_c |g_c| (abs on ScalarE, adds on DVE);  S = sum_c g_c on
        # GpSimd (idle during precompute, keeps DVE off the critical path)
        nc.scalar.activation(A[:], gates[0][:], mybir.ActivationFunctionType.Abs)
        nc.gpsimd.tensor_tensor(S[:], gates[0][:], gates[1][:], op=ALU.add)
        for c in range(1, 8):
            abc = tmp_pool.tile([P, FLAT], F32, tag="abst", name="abst_sb", bufs=2)
            nc.scalar.activation(abc[:], gates[c][:], mybir.ActivationFunctionType.Abs)
            nc.vector.tensor_tensor(A[:], A[:], abc[:], op=ALU.add)
            if c >= 2:
                nc.gpsimd.tensor_tensor(S[:], S[:], gates[c][:], op=ALU.add)
        # A = 1 / max(A, EPS)
        nc.vector.tensor_scalar_max(A[:], A[:], EPS)
        nc.vector.reciprocal(A[:], A[:])
        # gates *= 1/A ; gs = S/A ; base = (1 - gs) * blur
        for c in range(8):
            nc.vector.tensor_tensor(gates[c][:], gates[c][:], A[:], op=ALU.mult)
        nc.vector.tensor_tensor(S[:], S[:], A[:], op=ALU.mult)
        nc.vector.tensor_scalar(S[:], S[:], -1.0, 1.0, op0=ALU.mult, op1=ALU.add)
        d_center = rviews[0][:, 1:1 + RP, 1:1 + W]
        bview = base[:].rearrange("p (j w) -> p j w", j=RP)
        nc.vector.tensor_tensor(bview, S[:].rearrange("p (j w) -> p j w", j=RP),
                                d_center, op=ALU.mult)

    # ---- iteration loop ----
    # act_chunked: PSUM laid out as 8 banks of 512 f32, each bank holding a
    # 320-element chunk (bank-aligned so each matmul accumulates within one
    # bank). Two chunks cover one image row (640), so the PSUM->SBUF copy-out
    # runs per row slot, pipelined right behind the PE. Otherwise: 5 chunks
    # of 512 and one big copy-out.
    if act_chunked:
        MMCH, BANK, NMM = 320, 512, 8
    else:
        MMCH, BANK, NMM = 512, 512, 5
    prod_pool = ctx.enter_context(tc.tile_pool(name="prod", bufs=prod_bufs))
    psum_pool = ctx.enter_context(tc.tile_pool(name="acc", bufs=1, space="PSUM"))

    def mm_plane(psum, plane, start, stop):
        for q in range(NMM):
            nc.tensor.matmul(psum[:, q * BANK:q * BANK + MMCH], ident[:],
                             plane[:, q * MMCH:(q + 1) * MMCH],
                             start=start, stop=stop)

    gp_set = ORDERED_C[:gp_products]          # dy==0 channels go to GpSimd
    dve_set = ORDERED_C[gp_products:n_products]  # < 8 is a timing-only variant

    out_t = out_d.rearrange("(p j) w -> p j w", j=RP)
    niter = prop_time * repeat
    cur, nxt = 0, 1
    for it in range(niter):
        final = it == niter - 1
        rv = rviews[cur]
        psum = psum_pool.tile([P, NMM * BANK], F32, tag="psum", name="psum_t")

        def product(c, eng, split):
            dy, dx = OFFSETS[c]
            pr = prod_pool.tile([P, FLAT], F32R, tag="prod", name="prod_t")
            pv_ = pr[:].rearrange("p (j w) -> p j w", j=RP)
            rin = rv[:, 1 + dy:1 + dy + RP, 1 + dx:1 + dx + W]
            if split:
                # per-row-slot sub-ops so the op can chase the ACT copy chain
                for j in range(RP):
                    eng.tensor_tensor(pv_[:, j:j + 1], gviews[c][:, j:j + 1],
                                      rin[:, j:j + 1], op=ALU.mult)
            else:
                eng.tensor_tensor(pv_, gviews[c], rin, op=ALU.mult)
            return pr

        if fuse_groups:
            mm_plane(psum, base, True, False)      # base opens each group
            for gi, (chans, gt) in enumerate(GROUPS):
                ncg = len(chans)
                dy = OFFSETS[chans[0]][0]
                dx0 = OFFSETS[chans[0]][1]
                dxs = OFFSETS[chans[1]][1] - dx0   # dx step within group
                pr = prod_pool.tile([P, 3 * FLAT], F32R, tag="prod",
                                    name="prod_t", bufs=2)
                pv_ = pr[:, :ncg * FLAT].rearrange(
                    "p (c j w) -> p c j w", c=ncg, j=RP)
                gv_ = gt[:].rearrange("p (c j w) -> p c j w", c=ncg, j=RP)
                rin = rv[:, 1 + dy:1 + dy + RP, 1 + dx0:1 + dx0 + W]
                rin4 = _add_free_dim(rin, 1, dxs, ncg)
                if gi == 0:
                    # row-split so the op chases the ACT copy chain
                    for j in range(RP):
                        nc.vector.tensor_tensor(pv_[:, :, j:j + 1],
                                                gv_[:, :, j:j + 1],
                                                rin4[:, :, j:j + 1],
                                                op=ALU.mult)
                else:
                    nc.vector.tensor_tensor(pv_, gv_, rin4, op=ALU.mult)
                for ci in range(ncg):
                    last = gi == len(GROUPS) - 1 and ci == ncg - 1
                    mm_plane(psum, pr[:, ci * FLAT:(ci + 1) * FLAT],
                             False, last)
        else:
            # launch GpSimd products first (ready at iteration start), but
            # their matmul planes go right after base so the slow products
            # never gate the tail of the in-order PE queue.
            gp_prods = [product(c, nc.gpsimd, False) for c in gp_set]
            mm_plane(psum, base, True, False)      # base opens each group
            for pr in gp_prods:
                mm_plane(psum, pr, False, False)
            if split_first >= 2 and len(dve_set) >= 2:
                # row-split the first split_first products, interleaved, so
                # DVE has work the moment each ACT row-copy of the previous
                # iteration lands
                nsp = min(split_first, len(dve_set) - 1)
                cs = dve_set[:nsp]
                prs, pvs, rins = [], [], []
                for c in cs:
                    dy, dx = OFFSETS[c]
                    pr = prod_pool.tile([P, FLAT], F32R, tag="prod",
                                        name="prod_t")
                    prs.append(pr)
                    pvs.append(pr[:].rearrange("p (j w) -> p j w", j=RP))
                    rins.append(rv[:, 1 + dy:1 + dy + RP, 1 + dx:1 + dx + W])
                for j in range(RP):
                    for t, c in enumerate(cs):
                        nc.vector.tensor_tensor(
                            pvs[t][:, j:j + 1],
                            gviews[c][:, j:j + 1],
                            rins[t][:, j:j + 1], op=ALU.mult)
                for pr in prs:
                    mm_plane(psum, pr, False, False)
                rest = dve_set[nsp:]
            else:
                rest = dve_set[1:]
                pr = product(dve_set[0], nc.vector, split=bool(split_first))
                mm_plane(psum, pr, False, len(rest) == 0)
            for i, c in enumerate(rest):
                is_last = i == len(rest) - 1
                if split_last and is_last:
                    # row-split the LAST product and emit its bank matmuls per
                    # row, so the ACT copy chain (which needs banks 0,1 of all
                    # planes) starts while the remaining rows still compute --
                    # the iteration boundary then has no DVE idle.
                    dy, dx = OFFSETS[c]
                    pr = prod_pool.tile([P, FLAT], F32R, tag="prod",
                                        name="prod_t")
                    pv_ = pr[:].rearrange("p (j w) -> p j w", j=RP)
                    rin = rv[:, 1 + dy:1 + dy + RP, 1 + dx:1 + dx + W]
                    for j in range(RP):
                        nc.vector.tensor_tensor(pv_[:, j:j + 1],
                                                gviews[c][:, j:j + 1],
                                                rin[:, j:j + 1], op=ALU.mult)
                        for q in (2 * j, 2 * j + 1):
                            nc.tensor.matmul(
                                psum[:, q * BANK:q * BANK + MMCH], ident[:],
                                pr[:, q * MMCH:(q + 1) * MMCH],
                                start=False, stop=True)
                else:
                    pr = product(c, nc.vector, split=False)
                    mm_plane(psum, pr, False, is_last)
        nv = rviews[nxt]
        rbn = rbufs[nxt]
        if act_chunked:
            # PSUM -> next r center per row slot (ScalarE), halos via DMA.
            # Row slot j covers psum banks 2j, 2j+1.
            pv = psum[:].rearrange("p (q b) -> p q b", q=NMM)
            for j in range(RP):
                row = rbn[:, (1 + j) * ROWW + 1:(1 + j) * ROWW + 1 + W]
                nc.scalar.activation(row.rearrange("p (a b) -> p a b", a=2),
                                     pv[:, 2 * j:2 * j + 2, 0:MMCH],
                                     mybir.ActivationFunctionType.Copy)
                if final:
                    # last iteration: store each row group as soon as its
                    # copy lands (no halo refresh needed anymore)
                    nc.sync.dma_start(out=out_t[:, j:j + 1, :],
                                      in_=nv[:, 1 + j:2 + j, 1:1 + W])
                elif j == 0:
                    # down halo (slot RP+1 of p <- slot 1 of p+1): needs j0
                    nc.sync.dma_start(out=nv[0:P - 1, RP + 1:RP + 2, :],
                                      in_=nv[1:P, 1:2, :])
            if not final:
                # up halo (slot 0 of p <- slot RP of p-1): needs row j3
                nc.sync.dma_start(out=nv[1:P, 0:1, :],
                                  in_=nv[0:P - 1, RP:RP + 1, :])
        else:
            nc.scalar.activation(nv[:, 1:1 + RP, 1:1 + W],
                                 psum[:].rearrange("p (j w) -> p j w", j=RP),
                                 mybir.ActivationFunctionType.Copy)
            if final:
                nc.sync.dma_start(out=out_t, in_=nv[:, 1:1 + RP, 1:1 + W])
            else:
                nc.sync.dma_start(out=nv[0:P - 1, RP + 1:RP + 2, :],
                                  in_=nv[1:P, 1:2, :])
                nc.sync.dma_start(out=nv[1:P, 0:1, :],
                                  in_=nv[0:P - 1, RP:RP + 1, :])
        cur, nxt = nxt, cur

    if niter == 0:
        # no iterations: output is the loaded blur
        nc.sync.dma_start(out=out_t, in_=rviews[cur][:, 1:1 + RP, 1:1 + W])


_NC_CACHE = {}


def build_nc(prop_time: int, repeat: int = 1, gp_products: int = 0,
             fuse_groups: bool = False, act_chunked: bool = True,
             split_first: int = 2, n_products: int = 8, pre_repeat: int = 1,
             split_last: bool = False, prod_bufs: int = 6):
    key = (prop_time, repeat, gp_products, fuse_groups, act_chunked,
           split_first, n_products, pre_repeat, split_last, prod_bufs)
    if key in _NC_CACHE:
        return _NC_CACHE[key]
    nc = bacc.Bacc("TRN2", target_bir_lowering=False, debug=False)
    guid = nc.dram_tensor("guidance", [8, H, W], F32, kind="ExternalInput").ap()
    blur = nc.dram_tensor("blur", [H, W], F32, kind="ExternalInput").ap()
    ident_d = nc.dram_tensor("ident", [P, P], F32R, kind="ExternalInput").ap()
    out_d = nc.dram_tensor("out", [H, W], F32, kind="ExternalOutput").ap()
    with tile.TileContext(nc) as tc, ExitStack() as ctx:
        _emit(ctx, tc, guid, blur, ident_d, out_d, prop_time, repeat,
              gp_products=gp_products, fuse_groups=fuse_groups,
              act_chunked=act_chunked, split_first=split_first,
              n_products=n_products, pre_repeat=pre_repeat,
              split_last=split_last, prod_bufs=prod_bufs)
    nc.compile()
    _NC_CACHE[key] = nc
    return nc


def make_in_maps(guidance: np.ndarray, blur_depth: np.ndarray):
    eye = np.eye(P, dtype=np.float32)
    return [
        {
            "guidance": np.ascontiguousarray(guidance[b], dtype=np.float32),
            "blur": np.ascontiguousarray(blur_depth[b, 0], dtype=np.float32),
            "ident": eye,
        }
        for b in range(B)
    ]


def kernel(guidance, blur_depth, prop_time):
    guidance = np.asarray(guidance, dtype=np.float32)
    blur_depth = np.asarray(blur_depth, dtype=np.float32)
    pt = int(np.asarray(prop_time))
    nc = build_nc(pt)
    in_maps = make_in_maps(guidance, blur_depth)
    res = run_bass_kernel_spmd(nc, in_maps, list(range(NCORES)))
    out = np.stack([res.results[b]["out"] for b in range(B)])[:, None]
    return out.astype(np.float32)



# revision 3
# speedup vs baseline: 1.8396x; 1.8396x over previous
"""Trainium2 Bass kernel for nn_Affinity_Propagate — fp16 iteration state.

Algorithm (per batch image, one image per NeuronCore, 8 cores data-parallel):
    gate_wb[c] = shift_c(guidance[c])           (SPN shift, zero pad)
    w[c]       = gate_wb[c] / max(sum_c |gate_wb[c]|, eps)
    base       = (1 - sum_c w[c]) * blur
    r          = blur;  repeat prop_time times:  r = sum_c w[c]*shift_c(r) + base

Same structure as the fp32 baseline (rows on 120 SBUF partitions, 4 per
partition with 1-row/col zero halos; DVE products, PE identity-matmul
accumulation into PSUM fp32, ACT PSUM->SBUF copy-out, halo rows refreshed by
two partition-shifted SBUF DMAs per iteration), but the whole iteration
state (gates, r, products, base) is float16, which roughly doubles DVE
tensor_tensor throughput (2x_1P packed mode) — the baseline's bottleneck.

fp16 packing needs every DVE operand 4B-aligned with step-1 innermost runs,
so products cannot read r at odd column offsets (the +-1 column shifts of
the stencil). Instead each gate tile bakes its channel's column shift: gate
tiles are row-padded [P, RP, ROWW] with w_c placed at column offset 1+dx,
products multiply the FULL padded rows (flat [P, RP*ROWW], r read at offset
(1+dy)*ROWW -- always even), and the PE's rhs access pattern reads the
product plane at column offset 1+dx so values land at the right output
position. Out-of-range taps are zero because the r tile's halo columns stay
zero and gate borders are zeroed at load. Guidance/blur are uploaded as
fp16 (halves the HBM load traffic); the output is downloaded as fp16 and
cast to fp32 on host.

Measured (axon trn2, R-slope): 24-iteration loop ~118-120 us (~4.9 us/iter,
vs ~420 us for the fp32 baseline); absmax relative error ~9.7e-4 vs the
fp32 reference (fp16 gate/product/state rounding; tolerance is 2e-2).

build_nc knobs: repeat=N repeats the iteration loop in-NEFF (timing slopes);
whole_repeat=N repeats the entire body incl. loads+precompute (whole-kernel
slope); fuse=True merges each dy-group's products into one DVE op via a
step-0 operand dim (measured slower on HW than 8 separate ops -- the packed
mode appears not to engage for those APs -- so default False).
"""

import numpy as np
from contextlib import ExitStack

import concourse.bacc as bacc
import concourse.tile as tile
import concourse.mybir as mybir
from concourse.bass_utils import run_bass_kernel_spmd

H, W = 480, 640
B = 8
NCORES = 8
RP = 4                  # image rows per partition
P = H // RP             # 120 partitions used
NROW = RP + 2           # row slots incl. top/bottom halo
ROWW = W + 2            # row width incl. left/right halo (even)
FLAT = RP * W           # 2560 tight free elems per partition
FLATP = RP * ROWW       # 2568 padded free elems per partition
OFFSETS = [(-1, -1), (-1, 0), (-1, 1), (0, -1), (0, 1), (1, -1), (1, 0), (1, 1)]
EPS = 1e-4

F16 = mybir.dt.float16
F32 = mybir.dt.float32
ALU = mybir.AluOpType
ACTF = mybir.ActivationFunctionType

# product emission order: dy==0 first (no row-halo dep), then dy==+1 (down
# halo, written first), then dy==-1 (up halo, written last)
ORDERED_C = [3, 4, 5, 6, 7, 0, 1, 2]
GROUPS = [(0, [3, 4]), (1, [5, 6, 7]), (-1, [0, 1, 2])]   # (dy, channels)

MMCH, BANK, NMM = 320, 512, 8   # PSUM: 8 banks, 320-elem chunk per bank


def _rep_dim(ap, count):
    """Prefix a [step=0, count] free dim so one DVE op re-reads the same
    operand for each channel of a group."""
    ap = ap.unsqueeze(1).copy()
    ap.ap[1] = [0, count]
    return ap


def _load_shifted(nc, g_view, src2d, dy, dx):
    """DMA guidance channel (as [H, W] dram AP) shifted by (dy, dx) into the
    tight gate tile view g_view ([P, RP, W]); border elements are left
    untouched (pre-zeroed)."""
    r0 = max(0, -dy)            # dest flat row range [r0, r1)
    r1 = H - max(0, dy)
    x0 = max(0, -dx)            # dest col range [x0, x1)
    x1 = W - max(0, dx)
    p_start = (r0 + RP - 1) // RP
    p_end = r1 // RP
    if p_end > p_start:
        src = src2d[RP * p_start + dy:RP * p_end + dy, x0 + dx:x1 + dx]
        src = src.rearrange("(p j) w -> p j w", j=RP)
        nc.sync.dma_start(out=g_view[p_start:p_end, :, x0:x1], in_=src)
    if r0 % RP != 0:
        p = p_start - 1
        j0 = r0 - RP * p
        src = src2d[RP * p + j0 + dy:RP * (p + 1) + dy, x0 + dx:x1 + dx]
        src = src.rearrange("(p j) w -> p j w", j=RP - j0)
        nc.sync.dma_start(out=g_view[p:p + 1, j0:RP, x0:x1], in_=src)
    if r1 % RP != 0:
        p = p_end
        j1 = r1 - RP * p
        src = src2d[RP * p + dy:RP * p + j1 + dy, x0 + dx:x1 + dx]
        src = src.rearrange("(p j) w -> p j w", j=j1)
        nc.sync.dma_start(out=g_view[p:p + 1, 0:j1, x0:x1], in_=src)


def _emit(ctx, tc, guid, blur, ident_d, out_d, prop_time, repeat=1,
          split_first=2, fuse=False):
    nc = tc.nc

    const_pool = ctx.enter_context(tc.tile_pool(name="const", bufs=1))
    r_pool = ctx.enter_context(tc.tile_pool(name="rbuf", bufs=1))

    ident = const_pool.tile([P, P], F16, tag="ident", name="ident_sb")
    nc.sync.dma_start(out=ident[:], in_=ident_d)

    # baked gate tiles: w_c at column offset 1+dx within padded rows.
    # When fused, the channels of each dy-group live in one contiguous tile
    # so the group's products are a single DVE op.
    if fuse:
        grp_tiles = [
            const_pool.tile([P, len(ch) * FLATP], F16, tag=f"gg{gi}",
                            name=f"gg{gi}_sb")
            for gi, (dy, ch) in enumerate(GROUPS)
        ]
        gates = [None] * 8
        for gt, (dy, chans) in zip(grp_tiles, GROUPS):
            for ci, c in enumerate(chans):
                gates[c] = gt[:, ci * FLATP:(ci + 1) * FLATP]
        gate_mem = grp_tiles
    else:
        gtiles = [const_pool.tile([P, FLATP], F16, tag=f"g{c}", name=f"g{c}_sb")
                  for c in range(8)]
        gates = [t[:] for t in gtiles]
        gate_mem = gtiles
    base = const_pool.tile([P, FLAT], F16, tag="base", name="base_sb")

    rbufs = [r_pool.tile([P, NROW * ROWW], F16, tag=f"r{i}", name=f"r{i}_sb")
             for i in range(2)]
    for rb in rbufs:
        nc.gpsimd.memset(rb[:], 0.0)
    for g in gate_mem:
        nc.gpsimd.memset(g[:], 0.0)

    rviews = [rb[:].rearrange("p (r w) -> p r w", r=NROW) for rb in rbufs]

    # blur -> r0 center, then initial row-halo exchange
    blur_t = blur.rearrange("(p j) w -> p j w", j=RP)
    rv0 = rviews[0]
    nc.sync.dma_start(out=rv0[:, 1:1 + RP, 1:1 + W], in_=blur_t)
    nc.sync.dma_start(out=rv0[1:P, 0:1, :], in_=rv0[0:P - 1, RP:RP + 1, :])
    nc.sync.dma_start(out=rv0[0:P - 1, RP + 1:RP + 2, :], in_=rv0[1:P, 1:2, :])

    # ---- one-time precompute -------------------------------------------
    with tc.tile_pool(name="pretmp", bufs=1) as tmp_pool, \
         tc.tile_pool(name="prepsum", bufs=1, space="PSUM") as ppsum_pool:
        # tight SPN-shifted guidance loads
        gts = []
        for c, (dy, dx) in enumerate(OFFSETS):
            gt = tmp_pool.tile([P, FLAT], F16, tag=f"gt{c}", name=f"gt{c}_sb")
            gv = gt[:].rearrange("p (j w) -> p j w", j=RP)
            if dy == -1:
                nc.vector.memset(gv[:, 0:1, :], 0.0)
            elif dy == 1:
                nc.vector.memset(gv[:, RP - 1:RP, :], 0.0)
            if dx == -1:
                nc.vector.memset(gv[:, :, 0:1], 0.0)
            elif dx == 1:
                nc.vector.memset(gv[:, :, W - 1:W], 0.0)
            _load_shifted(nc, gv, guid[c], dy, dx)
            gts.append(gt)

        # S = sum_c g_c on the (idle) PE via identity matmuls
        spsum = ppsum_pool.tile([P, NMM * BANK], F32, tag="spsum", name="spsum_t")
        for ci, gt in enumerate(gts):
            for q in range(NMM):
                nc.tensor.matmul(spsum[:, q * BANK:q * BANK + MMCH], ident[:],
                                 gt[:, q * MMCH:(q + 1) * MMCH],
                                 start=(ci == 0), stop=(ci == 7))
        S = tmp_pool.tile([P, FLAT], F16, tag="rawsum", name="rawsum_sb")
        nc.scalar.activation(
            S[:].rearrange("p (q b) -> p q b", q=NMM),
            spsum[:].rearrange("p (q b) -> p q b", q=NMM)[:, :, 0:MMCH],
            ACTF.Copy)

        # A = sum_c |g_c| (abs on ScalarE, adds on DVE)
        A = tmp_pool.tile([P, FLAT], F16, tag="absum", name="absum_sb")
        nc.scalar.activation(A[:], gts[0][:], ACTF.Abs)
        for c in range(1, 8):
            abc = tmp_pool.tile([P, FLAT], F16, tag="abst", name="abst_sb", bufs=2)
            nc.scalar.activation(abc[:], gts[c][:], ACTF.Abs)
            nc.vector.tensor_tensor(A[:], A[:], abc[:], op=ALU.add)
        nc.vector.tensor_scalar_max(A[:], A[:], EPS)
        nc.vector.reciprocal(A[:], A[:])

        # baked gates: w'_c[:, :, 1+dx : 1+dx+W] = g_c * (1/A)
        Av = A[:].rearrange("p (j w) -> p j w", j=RP)
        for c, (dy, dx) in enumerate(OFFSETS):
            gw = gates[c].rearrange("p (j w) -> p j w", j=RP)
            nc.vector.tensor_tensor(
                gw[:, :, 1 + dx:1 + dx + W],
                gts[c][:].rearrange("p (j w) -> p j w", j=RP),
                Av, op=ALU.mult)

        # base = (1 - S/A) * blur
        nc.vector.tensor_tensor(S[:], S[:], A[:], op=ALU.mult)
        nc.vector.tensor_scalar(S[:], S[:], -1.0, 1.0, op0=ALU.mult, op1=ALU.add)
        d_center = rviews[0][:, 1:1 + RP, 1:1 + W]
        bview = base[:].rearrange("p (j w) -> p j w", j=RP)
        nc.vector.tensor_tensor(bview, S[:].rearrange("p (j w) -> p j w", j=RP),
                                d_center, op=ALU.mult)

    # ---- iteration loop -------------------------------------------------
    prod_pool = ctx.enter_context(
        tc.tile_pool(name="prod", bufs=5 if fuse else 6))
    psum_pool = ctx.enter_context(tc.tile_pool(name="acc", bufs=1, space="PSUM"))

    def mm_plane(psum, plane, dx, start, stop):
        """Accumulate a padded product plane into psum, reading the plane at
        column offset 1+dx so values land at the right output position.
        plane is [P, FLATP]; chunk q covers row j=q//2, cols [320*(q%2), +320)."""
        for q in range(NMM):
            j, h = q // 2, q % 2
            off = j * ROWW + 1 + dx + h * MMCH
            nc.tensor.matmul(psum[:, q * BANK:q * BANK + MMCH], ident[:],
                             plane[:, off:off + MMCH],
                             start=start, stop=stop)

    def mm_tight(psum, plane, start, stop):
        for q in range(NMM):
            nc.tensor.matmul(psum[:, q * BANK:q * BANK + MMCH], ident[:],
                             plane[:, q * MMCH:(q + 1) * MMCH],
                             start=start, stop=stop)

    out_t = out_d.rearrange("(p j) w -> p j w", j=RP)
    niter = prop_time * repeat
    cur, nxt = 0, 1
    for it in range(niter):
        final = it == niter - 1
        rb = rbufs[cur]
        psum = psum_pool.tile([P, NMM * BANK], F32, tag="psum", name="psum_t")

        mm_tight(psum, base, True, False)       # base opens each bank group

        if fuse:
            # one DVE op per dy-group; the r operand gets a step-0 outer dim
            for gi, (dy, chans) in enumerate(GROUPS):
                ncg = len(chans)
                pr = prod_pool.tile([P, 3 * FLATP], F16, tag="prod",
                                    name="prod_t")
                pv = pr[:, :ncg * FLATP].rearrange("p (c f) -> p c f", c=ncg)
                gv = gate_mem[gi][:].rearrange("p (c f) -> p c f", c=ncg)
                if gi == 0 and split_first:
                    # row-split the dy==0 group so DVE has work the moment
                    # each ACT row-copy of the previous iteration lands
                    for j in range(RP):
                        sl = slice(j * ROWW, (j + 1) * ROWW)
                        rsl = rb[:, (1 + j) * ROWW:(2 + j) * ROWW]
                        nc.vector.tensor_tensor(pv[:, :, sl], gv[:, :, sl],
                                                _rep_dim(rsl, ncg), op=ALU.mult)
                else:
                    rsl = rb[:, (1 + dy) * ROWW:(1 + dy) * ROWW + FLATP]
                    nc.vector.tensor_tensor(pv, gv, _rep_dim(rsl, ncg),
                                            op=ALU.mult)
                for ci, c in enumerate(chans):
                    last = gi == len(GROUPS) - 1 and ci == ncg - 1
                    mm_plane(psum, pr[:, ci * FLATP:(ci + 1) * FLATP],
                             OFFSETS[c][1], False, last)
        else:
            def product(c):
                dy, _ = OFFSETS[c]
                pr = prod_pool.tile([P, FLATP], F16, tag="prod", name="prod_t")
                nc.vector.tensor_tensor(
                    pr[:], gates[c],
                    rb[:, (1 + dy) * ROWW:(1 + dy) * ROWW + FLATP],
                    op=ALU.mult)
                return pr

            if split_first >= 2:
                cs = ORDERED_C[:2]
                prs = [prod_pool.tile([P, FLATP], F16, tag="prod", name="prod_t")
                       for _ in cs]
                for j in range(RP):
                    for t, c in enumerate(cs):
                        sl = slice(j * ROWW, (j + 1) * ROWW)
                        rsl = slice((1 + j) * ROWW, (2 + j) * ROWW)
                        nc.vector.tensor_tensor(prs[t][:, sl], gates[c][:, sl],
                                                rb[:, rsl], op=ALU.mult)
                for t, c in enumerate(cs):
                    mm_plane(psum, prs[t], OFFSETS[c][1], False, False)
                rest = ORDERED_C[2:]
            else:
                rest = ORDERED_C

            for i, c in enumerate(rest):
                is_last = i == len(rest) - 1
                pr = product(c)
                mm_plane(psum, pr, OFFSETS[c][1], False, is_last)

        nv = rviews[nxt]
        rbn = rbufs[nxt]
        # PSUM -> next r center per row slot (ScalarE), halos via DMA.
        pv = psum[:].rearrange("p (q b) -> p q b", q=NMM)
        for j in range(RP):
            row = rbn[:, (1 + j) * ROWW + 1:(1 + j) * ROWW + 1 + W]
            nc.scalar.activation(row.rearrange("p (a b) -> p a b", a=2),
                                 pv[:, 2 * j:2 * j + 2, 0:MMCH],
                                 ACTF.Copy)
            if final:
                nc.sync.dma_start(out=out_t[:, j:j + 1, :],
                                  in_=nv[:, 1 + j:2 + j, 1:1 + W])
            elif j == 0:
                nc.sync.dma_start(out=nv[0:P - 1, RP + 1:RP + 2, :],
                                  in_=nv[1:P, 1:2, :])
        if not final:
            nc.sync.dma_start(out=nv[1:P, 0:1, :],
                              in_=nv[0:P - 1, RP:RP + 1, :])
        cur, nxt = nxt, cur

    if niter == 0:
        nc.sync.dma_start(out=out_t, in_=rviews[cur][:, 1:1 + RP, 1:1 + W])


_NC_CACHE = {}


def build_nc(prop_time: int, repeat: int = 1, split_first: int = 2,
             fuse: bool = False, whole_repeat: int = 1):
    key = (prop_time, repeat, split_first, fuse, whole_repeat)
    if key in _NC_CACHE:
        return _NC_CACHE[key]
    nc = bacc.Bacc("TRN2", target_bir_lowering=False, debug=False)
    guid = nc.dram_tensor("guidance", [8, H, W], F16, kind="ExternalInput").ap()
    blur = nc.dram_tensor("blur", [H, W], F16, kind="ExternalInput").ap()
    ident_d = nc.dram_tensor("ident", [P, P], F16, kind="ExternalInput").ap()
    out_d = nc.dram_tensor("out", [H, W], F16, kind="ExternalOutput").ap()
    with tile.TileContext(nc) as tc, \
            nc.allow_low_precision(reason="fp16 state; tol 2e-2, measured ~1e-3"):
        for _ in range(whole_repeat):
            with ExitStack() as ctx:
                _emit(ctx, tc, guid, blur, ident_d, out_d, prop_time, repeat,
                      split_first=split_first, fuse=fuse)
    nc.compile()
    _NC_CACHE[key] = nc
    return nc


def make_in_maps(guidance: np.ndarray, blur_depth: np.ndarray):
    eye = np.eye(P, dtype=np.float16)
    return [
        {
            "guidance": np.ascontiguousarray(guidance[b], dtype=np.float16),
            "blur": np.ascontiguousarray(blur_depth[b, 0], dtype=np.float16),
            "ident": eye,
        }
        for b in range(B)
    ]


def kernel(guidance, blur_depth, prop_time):
    guidance = np.asarray(guidance, dtype=np.float32)
    blur_depth = np.asarray(blur_depth, dtype=np.float32)
    pt = int(np.asarray(prop_time))
    nc = build_nc(pt)
    in_maps = make_in_maps(guidance, blur_depth)
    res = run_bass_kernel_spmd(nc, in_maps, list(range(NCORES)))
    out = np.stack([res.results[b]["out"] for b in range(B)])[:, None]
    return out.astype(np.float32)
